# revision 2
# baseline (speedup 1.0000x reference)
# Trainium2 Bass kernel for nn_AttentionBlock (GroupNorm + single-head
# self-attention over 32x32 spatial, C=512) — data-parallel over batch:
# 8 batch elements -> 8 NeuronCores, weights replicated.
#
# fp8(e4m3) DoubleRow design: all big matmuls run as fp8 DoubleRow
# (256-deep contraction per instruction, 0.5 cyc/row), scores are
# computed transposed (k stationary) so no PE transposes are needed,
# softmax row-sums come from a ones-matrix matmul broadcast to all
# partitions, and the normalization is folded into the attn@V eviction.
# Scale bookkeeping: weights x64 on host, activations stored x4 in fp8,
# descale factors folded into the PSUM evictions.
import numpy as np

CH = 512          # channels
N = 1024          # spatial H*W = 32*32
P = 128           # SBUF partitions
KT = CH // P      # 4 channel tiles
MT = N // P       # 8 spatial tiles (keys)
GROUPS = 8        # groupnorm groups (64 channels each)
EPS = 1e-5
SCALE = 1.0 / np.sqrt(CH)
NCORES = 8

_CACHE = {}


def _build_bass():
    import concourse.bacc as bacc
    import concourse.tile as tile
    from concourse import mybir

    f32 = mybir.dt.float32
    f8 = mybir.dt.float8e4
    Act = mybir.ActivationFunctionType
    Alu = mybir.AluOpType
    DR = mybir.MatmulPerfMode.DoubleRow

    nc = bacc.Bacc("TRN2")

    x_d = nc.dram_tensor("x", [CH, N], f32, kind="ExternalInput")
    wq_d = nc.dram_tensor("wq8", [P, KT, CH], f8, kind="ExternalInput")
    wk_d = nc.dram_tensor("wk8", [P, KT, CH], f8, kind="ExternalInput")
    wv_d = nc.dram_tensor("wv8", [P, KT, CH], f8, kind="ExternalInput")
    wp_d = nc.dram_tensor("wp8", [P, KT, CH], f8, kind="ExternalInput")
    # packed per-channel vectors: cols = 4*bq|4*bk|gnw|gnb|bp' (4 each)
    vec_d = nc.dram_tensor("vecs", [P, 20], f32, kind="ExternalInput")
    avg_d = nc.dram_tensor("avg", [P, P], f32, kind="ExternalInput")
    y_d = nc.dram_tensor("y", [CH, N], f32, kind="ExternalOutput")

    with tile.TileContext(nc) as tc:
        with (
            tc.tile_pool(name="persist", bufs=1) as persist,
            tc.tile_pool(name="work", bufs=2) as work,
            tc.tile_pool(name="small", bufs=2) as small,
        ):
            # ---- persistent SBUF tensors ----
            x_sb = persist.tile([P, KT, N], f32, tag="x")
            n_sb = persist.tile([P, KT, N], f8, tag="n")
            q_sb = persist.tile([P, KT, N], f8, tag="q")
            k_sb = persist.tile([P, KT, N], f8, tag="k")
            vT_sb = persist.tile([P, MT, CH], f8, tag="vT")
            eT_sb = persist.tile([P, MT, N], f8, tag="eT")
            o_sb = persist.tile([P, KT, N], f8, tag="o")
            rsr_sb = persist.tile([P, N], f32, tag="rsr")
            xbp_sb = persist.tile([P, KT, N], f32, tag="xbp")
            wq_sb = persist.tile([P, KT, CH], f8, tag="wq")
            wk_sb = persist.tile([P, KT, CH], f8, tag="wk")
            wv_sb = persist.tile([P, KT, CH], f8, tag="wv")
            wp_sb = persist.tile([P, KT, CH], f8, tag="wp")
            vec_sb = persist.tile([P, 20], f32, tag="vecs")
            avg_sb = persist.tile([P, P], f32, tag="avg")
            ones_sb = persist.tile([P, 2, P], f8, tag="ones")
            warm_sb = persist.tile([P, 2, 512], f8, tag="warm")
            scr_sb = persist.tile([P, N], f32, tag="scr")
            zero_sb = persist.tile([P, 1], f32, tag="zero")
            eps_sb = persist.tile([P, 1], f32, tag="eps")
            dummy_sb = persist.tile([P, 1], f32, tag="dummy")
            bq_sb = vec_sb[:, 0:4]     # 4*q_b
            bk_sb = vec_sb[:, 4:8]     # 4*k_b
            gnw_sb = vec_sb[:, 8:12]
            gnb_sb = vec_sb[:, 12:16]
            bp_sb = vec_sb[:, 16:20]   # p_w @ v_b + p_b

            # constants + ACT sqrt-table preload while DMAs stream
            nc.vector.memset(zero_sb, 0.0)
            nc.vector.memset(eps_sb, EPS)
            nc.vector.memset(dummy_sb, 1.0)
            nc.gpsimd.memset(ones_sb, 1.0)
            nc.gpsimd.memset(warm_sb, 1.0)
            nc.scalar.activation(out=dummy_sb, in_=dummy_sb, func=Act.Sqrt,
                                 bias=zero_sb, scale=1.0)

            # ---- loads, ordered by first use ----
            nc.sync.dma_start(out=x_sb[:, 0, 0:512], in_=x_d[0:P, 0:512])
            nc.sync.dma_start(out=x_sb[:, 0, 512:1024], in_=x_d[0:P, 512:1024])
            nc.sync.dma_start(out=vec_sb[:], in_=vec_d[:])
            nc.sync.dma_start(out=avg_sb[:], in_=avg_d[:])
            for kt in range(1, KT):
                nc.sync.dma_start(out=x_sb[:, kt, :], in_=x_d[kt * P:(kt + 1) * P, :])
            nc.sync.dma_start(out=wq_sb[:], in_=wq_d[:])
            nc.sync.dma_start(out=wk_sb[:], in_=wk_d[:])
            nc.sync.dma_start(out=wv_sb[:], in_=wv_d[:])
            nc.sync.dma_start(out=wp_sb[:], in_=wp_d[:])

            with tc.tile_pool(name="ps_g", bufs=2, space="PSUM") as ps_g:
                # PE warmth: cheap fp8 DoubleRow matmuls into a scratch bank
                warm_ps = ps_g.tile([P, 512], f32, tag="warmps")

                def warm(k):
                    for _ in range(k):
                        nc.tensor.matmul(warm_ps, ones_sb[:], warm_sb[:],
                                         start=True, stop=True, perf_mode=DR)

                warm(14)

                # ---- GroupNorm: per-channel (mean, E[x^2]) -> group stats
                # via block-averaging matmul -> scale/shift -> n in fp8 ----
                for kt in range(KT):
                    st = small.tile([P, 2], f32, tag="st")  # mean | E[x^2]
                    if kt < 2:  # DVE bn_stats path
                        bstats = small.tile([P, 2, 6], f32, tag="bstats")
                        mv = small.tile([P, 2], f32, tag="mv")
                        nc.vector.bn_stats(out=bstats[:, 0, :], in_=x_sb[:, kt, 0:512])
                        nc.vector.bn_stats(out=bstats[:, 1, :], in_=x_sb[:, kt, 512:1024])
                        nc.vector.bn_aggr(out=mv, in_=bstats)
                        nc.vector.tensor_copy(st[:, 0:1], mv[:, 0:1])
                        nc.vector.scalar_tensor_tensor(
                            out=st[:, 1:2], in0=mv[:, 0:1], scalar=mv[:, 0:1],
                            in1=mv[:, 1:2], op0=Alu.mult, op1=Alu.add,
                        )
                    else:  # ACT accumulator path
                        nc.scalar.activation(out=scr_sb, in_=x_sb[:, kt, :],
                                             func=Act.Identity, bias=zero_sb,
                                             scale=1.0 / N, accum_out=st[:, 0:1])
                        nc.scalar.activation(out=scr_sb, in_=x_sb[:, kt, :],
                                             func=Act.Square, bias=zero_sb,
                                             scale=1.0 / np.sqrt(N),
                                             accum_out=st[:, 1:2])

                    # group aggregate+broadcast in one small matmul
                    b_ps = ps_g.tile([P, 2], f32, tag="gmm")
                    nc.tensor.matmul(b_ps, avg_sb, st, start=True, stop=True)
                    warm(8)
                    bc = small.tile([P, 2], f32, tag="bc")
                    nc.scalar.copy(bc, b_ps)
                    mean = bc[:, 0:1]
                    vneg = small.tile([P, 1], f32, tag="vneg")
                    nc.vector.scalar_tensor_tensor(
                        out=vneg, in0=mean, scalar=mean, in1=bc[:, 1:2],
                        op0=Alu.mult, op1=Alu.subtract,  # mean^2 - E[x^2]
                    )
                    var = small.tile([P, 1], f32, tag="var")
                    nc.vector.tensor_scalar_mul(var, vneg, -1.0)
                    sd = small.tile([P, 1], f32, tag="sd")
                    nc.scalar.activation(out=sd, in_=var, func=Act.Sqrt,
                                         bias=eps_sb, scale=1.0)
                    rstd = small.tile([P, 1], f32, tag="rstd")
                    nc.vector.reciprocal(rstd, sd)
                    gsc = small.tile([P, 1], f32, tag="gsc")
                    nc.vector.tensor_mul(gsc, rstd, gnw_sb[:, kt:kt + 1])
                    gshn = small.tile([P, 1], f32, tag="gshn")  # mean*gsc - gnb
                    nc.vector.scalar_tensor_tensor(
                        out=gshn, in0=mean, scalar=gsc, in1=gnb_sb[:, kt:kt + 1],
                        op0=Alu.mult, op1=Alu.subtract,
                    )
                    nc.vector.tensor_scalar(
                        out=n_sb[:, kt, :], in0=x_sb[:, kt, :],
                        scalar1=gsc, scalar2=gshn, op0=Alu.mult, op1=Alu.subtract,
                    )

                warm(8)
                # preload the exp table while the QKV matmuls stream
                nc.scalar.activation(out=dummy_sb, in_=x_sb[:, 0, 0:1], func=Act.Exp,
                                     bias=zero_sb, scale=1.0)

            # xbp = x + bp' (residual + folded proj/v bias), on GpSimd —
            # needed only at the proj evictions, computed in the background
            for dt in range(KT):
                nc.gpsimd.tensor_scalar_add(
                    xbp_sb[:, dt, :], x_sb[:, dt, :], bp_sb[:, dt:dt + 1])

            with tc.tile_pool(name="ps_qkv", bufs=4, space="PSUM") as ps_qkv:
                # ---- Q projection (DoubleRow, weights stationary) ----
                for dt in range(KT):
                    mm = ps_qkv.tile([P, N], f32, tag="mm", name=f"q{dt}")
                    for ktp in range(2):
                        for nh in range(2):
                            nc.tensor.matmul(
                                mm[:, nh * 512:(nh + 1) * 512],
                                wq_sb[:, 2 * ktp:2 * ktp + 2, dt * P:(dt + 1) * P],
                                n_sb[:, 2 * ktp:2 * ktp + 2, nh * 512:(nh + 1) * 512],
                                start=(ktp == 0), stop=(ktp == 1), perf_mode=DR,
                            )
                    # q8 = raw/16 + 4*bq  (= 4*q_true)
                    nc.scalar.activation(out=q_sb[:, dt, :], in_=mm,
                                         func=Act.Identity,
                                         bias=bq_sb[:, dt:dt + 1], scale=1.0 / 16)

                # ---- K projection: evict on DVE to balance engines ----
                for dt in range(KT):
                    mm = ps_qkv.tile([P, N], f32, tag="mm", name=f"k{dt}")
                    for ktp in range(2):
                        for nh in range(2):
                            nc.tensor.matmul(
                                mm[:, nh * 512:(nh + 1) * 512],
                                wk_sb[:, 2 * ktp:2 * ktp + 2, dt * P:(dt + 1) * P],
                                n_sb[:, 2 * ktp:2 * ktp + 2, nh * 512:(nh + 1) * 512],
                                start=(ktp == 0), stop=(ktp == 1), perf_mode=DR,
                            )
                    nc.vector.tensor_scalar(
                        out=k_sb[:, dt, :], in0=mm, scalar1=1.0 / 16,
                        scalar2=bk_sb[:, dt:dt + 1], op0=Alu.mult, op1=Alu.add)

                # ---- V transposed: vT[m, c] (n stationary, wv moving);
                # v bias folds into bp' on host ----
                for mg in range(KT):  # 2 m-tiles per psum tile
                    mm = ps_qkv.tile([P, 2, 512], f32, tag="mm", name=f"v{mg}")
                    for ml in range(2):
                        mt = 2 * mg + ml
                        for ktp in range(2):
                            nc.tensor.matmul(
                                mm[:, ml, :],
                                n_sb[:, 2 * ktp:2 * ktp + 2, mt * P:(mt + 1) * P],
                                wv_sb[:, 2 * ktp:2 * ktp + 2, :],
                                start=(ktp == 0), stop=(ktp == 1), perf_mode=DR,
                            )
                    nc.scalar.activation(out=vT_sb[:, 2 * mg:2 * mg + 2, :], in_=mm,
                                         func=Act.Identity, bias=zero_sb,
                                         scale=1.0 / 16)

            # ---- scores transposed + exp, pipelined per 2 m-tiles ----
            # sT[m, n] = sum_c k[c, m] q[c, n]; exp on ACT -> fp8 eT
            with tc.tile_pool(name="ps_s", bufs=2, space="PSUM") as ps_s:
                for mtp in range(4):
                    s_ps = ps_s.tile([P, 2, N], f32, tag="s", name=f"s{mtp}")
                    for ml in range(2):
                        mt = 2 * mtp + ml
                        for ktp in range(2):
                            for nh in range(2):
                                nc.tensor.matmul(
                                    s_ps[:, ml, nh * 512:(nh + 1) * 512],
                                    k_sb[:, 2 * ktp:2 * ktp + 2, mt * P:(mt + 1) * P],
                                    q_sb[:, 2 * ktp:2 * ktp + 2, nh * 512:(nh + 1) * 512],
                                    start=(ktp == 0), stop=(ktp == 1), perf_mode=DR,
                                )
                    # raw = 16*s_true; exp(SCALE/16 * raw) in [~0.1, ~8]
                    nc.scalar.activation(out=eT_sb[:, 2 * mtp:2 * mtp + 2, :],
                                         in_=s_ps, func=Act.Exp,
                                         bias=zero_sb, scale=SCALE / 16)

            with (
                tc.tile_pool(name="ps_sum", bufs=1, space="PSUM") as ps_sum,
                tc.tile_pool(name="ps_av", bufs=2, space="PSUM") as ps_av,
            ):
                # ---- softmax denominators, broadcast to all partitions ----
                sum_ps = ps_sum.tile([P, N], f32, tag="sum")
                for mtp in range(4):
                    for nh in range(2):
                        nc.tensor.matmul(
                            sum_ps[:, nh * 512:(nh + 1) * 512],
                            ones_sb[:],
                            eT_sb[:, 2 * mtp:2 * mtp + 2, nh * 512:(nh + 1) * 512],
                            start=(mtp == 0), stop=(mtp == 3), perf_mode=DR,
                        )
                nc.vector.reciprocal(rsr_sb, sum_ps)

                # ---- out[c, n] = (sum_m vT[m,c] eT[m,n]) / rowsum[n] ----
                for ct in range(KT):
                    mm = ps_av.tile([P, N], f32, tag="mm", name=f"av{ct}")
                    for mtp in range(4):
                        for nh in range(2):
                            nc.tensor.matmul(
                                mm[:, nh * 512:(nh + 1) * 512],
                                vT_sb[:, 2 * mtp:2 * mtp + 2, ct * P:(ct + 1) * P],
                                eT_sb[:, 2 * mtp:2 * mtp + 2, nh * 512:(nh + 1) * 512],
                                start=(mtp == 0), stop=(mtp == 3), perf_mode=DR,
                            )
                    # o8 = raw * rsr = 4*attnout_true
                    nc.vector.tensor_tensor(out=o_sb[:, ct, :], in0=mm,
                                            in1=rsr_sb, op=Alu.mult)

            with tc.tile_pool(name="ps_pr", bufs=2, space="PSUM") as ps_pr:
                # ---- final projection + residual, stream out ----
                for dt in range(KT):
                    mm = ps_pr.tile([P, N], f32, tag="mm", name=f"p{dt}")
                    for ktp in range(2):
                        for nh in range(2):
                            nc.tensor.matmul(
                                mm[:, nh * 512:(nh + 1) * 512],
                                wp_sb[:, 2 * ktp:2 * ktp + 2, dt * P:(dt + 1) * P],
                                o_sb[:, 2 * ktp:2 * ktp + 2, nh * 512:(nh + 1) * 512],
                                start=(ktp == 0), stop=(ktp == 1), perf_mode=DR,
                            )
                    y_sb = work.tile([P, N], f32, tag="y")
                    # y = raw/256 + (x + bp')
                    nc.vector.scalar_tensor_tensor(
                        out=y_sb, in0=mm, scalar=1.0 / 256,
                        in1=xbp_sb[:, dt, :], op0=Alu.mult, op1=Alu.add)
                    nc.sync.dma_start(out=y_d[dt * P:(dt + 1) * P, :], in_=y_sb)

    nc.finalize()
    return nc


def _get_nc():
    if "nc" not in _CACHE:
        _CACHE["nc"] = _build_bass()
    return _CACHE["nc"]


def _make_in_maps(x, gn_w, gn_b, q_w, q_b, k_w, k_b, v_w, v_b, p_w, p_b):
    import ml_dtypes
    f8 = ml_dtypes.float8_e4m3
    x = np.asarray(x, np.float32)
    B = x.shape[0]
    assert x.shape == (B, CH, 32, 32) and B == NCORES

    def pc(vec):  # [512] -> [128, 4] with c = t*128 + p
        return np.asarray(vec, np.float32).reshape(KT, P).T

    def w8(w):  # [Cout, Cin] -> fp8 [P, KT, Cout] of 64*w.T
        wt = np.asarray(w, np.float32).T * 64.0  # [Cin, Cout]
        return np.ascontiguousarray(
            wt.reshape(KT, P, CH).transpose(1, 0, 2).astype(f8))

    bp_fold = np.asarray(p_w, np.float32) @ np.asarray(v_b, np.float32) \
        + np.asarray(p_b, np.float32)
    vecs = np.concatenate(
        [pc(4.0 * np.asarray(q_b)), pc(4.0 * np.asarray(k_b)),
         pc(gn_w), pc(gn_b), pc(bp_fold)], axis=1
    )
    avg = np.kron(np.eye(2, dtype=np.float32),
                  np.full((64, 64), 1.0 / 64, np.float32))
    shared = {
        "wq8": w8(q_w),
        "wk8": w8(k_w),
        "wv8": w8(v_w),
        "wp8": w8(p_w),
        "vecs": np.ascontiguousarray(vecs),
        "avg": np.ascontiguousarray(avg),
    }
    return [
        dict(shared, x=np.ascontiguousarray(x[b].reshape(CH, N)))
        for b in range(B)
    ]


def _run(in_maps, **kwargs):
    from concourse.bass_utils import run_bass_kernel_spmd
    return run_bass_kernel_spmd(_get_nc(), in_maps, core_ids=list(range(NCORES)), **kwargs)


def kernel(**inputs):
    in_maps = _make_in_maps(**inputs)
    res = _run(in_maps)
    out = np.stack([r["y"].reshape(CH, 32, 32) for r in res.results], axis=0)
    return out.astype(np.float32)


# revision 3
# speedup vs baseline: 1.9994x; 1.9994x over previous
# Trainium2 Bass kernel for nn_AttentionBlock (GroupNorm + single-head
# self-attention over 32x32 spatial, C=512) — data-parallel over batch:
# 8 batch elements -> 8 NeuronCores, weights replicated.
#
# fp8(e4m3) DoubleRow design: all big matmuls run as fp8 DoubleRow
# (256-deep contraction per instruction, ~259ns/MM sustained), scores
# are computed transposed (k stationary) so no PE transposes are
# needed, softmax row-sums come from a ones-matrix matmul broadcast to
# all partitions, and the softmax normalization is folded into the
# attn@V eviction. Scale bookkeeping: weights x64 on host, activations
# stored x4 in fp8, descale factors folded into the PSUM evictions.
# Evictions are split ACT/DVE to balance engine load; GpSimd is not
# used for compute (it is ~25x slower and contends for DVE's SBUF port).
import numpy as np

CH = 512          # channels
N = 1024          # spatial H*W = 32*32
P = 128           # SBUF partitions
KT = CH // P      # 4 channel tiles
MT = N // P       # 8 spatial tiles (keys)
GROUPS = 8        # groupnorm groups (64 channels each)
EPS = 1e-5
SCALE = 1.0 / np.sqrt(CH)
NCORES = 8

_CACHE = {}


def _build_bass():
    import concourse.bacc as bacc
    import concourse.tile as tile
    from concourse import mybir

    f32 = mybir.dt.float32
    f8 = mybir.dt.float8e4
    Act = mybir.ActivationFunctionType
    Alu = mybir.AluOpType
    DR = mybir.MatmulPerfMode.DoubleRow

    nc = bacc.Bacc("TRN2")

    x_d = nc.dram_tensor("x", [CH, N], f32, kind="ExternalInput")
    wq_d = nc.dram_tensor("wq8", [P, KT, CH], f8, kind="ExternalInput")
    wk_d = nc.dram_tensor("wk8", [P, KT, CH], f8, kind="ExternalInput")
    wv_d = nc.dram_tensor("wv8", [P, KT, CH], f8, kind="ExternalInput")
    wp_d = nc.dram_tensor("wp8", [P, KT, CH], f8, kind="ExternalInput")
    # packed per-channel vectors: cols = 4*bq|4*bk|gnw|gnb|bp' (4 each)
    vec_d = nc.dram_tensor("vecs", [P, 20], f32, kind="ExternalInput")
    avg_d = nc.dram_tensor("avg", [P, P], f32, kind="ExternalInput")
    y_d = nc.dram_tensor("y", [CH, N], f32, kind="ExternalOutput")

    with tile.TileContext(nc) as tc:
        with (
            tc.tile_pool(name="persist", bufs=1) as persist,
            tc.tile_pool(name="work", bufs=2) as work,
            tc.tile_pool(name="small", bufs=2) as small,
        ):
            # ---- persistent SBUF tensors ----
            x_sb = persist.tile([P, KT, N], f32, tag="x")
            n_sb = persist.tile([P, KT, N], f8, tag="n")
            q_sb = persist.tile([P, KT, N], f8, tag="q")
            k_sb = persist.tile([P, KT, N], f8, tag="k")
            vT_sb = persist.tile([P, MT, CH], f8, tag="vT")
            eT_sb = persist.tile([P, MT, N], f8, tag="eT")
            o_sb = persist.tile([P, KT, N], f8, tag="o")
            rsr_sb = persist.tile([P, N], f32, tag="rsr")
            xbp_sb = persist.tile([P, KT, N], f32, tag="xbp")
            wq_sb = persist.tile([P, KT, CH], f8, tag="wq")
            wk_sb = persist.tile([P, KT, CH], f8, tag="wk")
            wv_sb = persist.tile([P, KT, CH], f8, tag="wv")
            wp_sb = persist.tile([P, KT, CH], f8, tag="wp")
            vec_sb = persist.tile([P, 20], f32, tag="vecs")
            avg_sb = persist.tile([P, P], f32, tag="avg")
            ones_sb = persist.tile([P, 2, P], f8, tag="ones")
            warm_sb = persist.tile([P, 2, P], f8, tag="warm")
            scr_sb = persist.tile([P, N], f32, tag="scr")
            zero_sb = persist.tile([P, 1], f32, tag="zero")
            eps_sb = persist.tile([P, 1], f32, tag="eps")
            dummy_sb = persist.tile([P, 1], f32, tag="dummy")
            bq_sb = vec_sb[:, 0:4]     # 4*q_b
            bk_sb = vec_sb[:, 4:8]     # 4*k_b
            gnw_sb = vec_sb[:, 8:12]
            gnb_sb = vec_sb[:, 12:16]
            bp_sb = vec_sb[:, 16:20]   # p_w @ v_b + p_b

            # constants + ACT sqrt-table preload while DMAs stream
            nc.vector.memset(zero_sb, 0.0)
            nc.vector.memset(eps_sb, EPS)
            nc.vector.memset(dummy_sb, 1.0)
            nc.vector.memset(ones_sb, 1.0)
            nc.vector.memset(warm_sb, 1.0)
            nc.scalar.activation(out=dummy_sb, in_=dummy_sb, func=Act.Sqrt,
                                 bias=zero_sb, scale=1.0)

            # ---- loads, ordered by first use ----
            nc.sync.dma_start(out=x_sb[:, 0, 0:512], in_=x_d[0:P, 0:512])
            nc.sync.dma_start(out=x_sb[:, 0, 512:1024], in_=x_d[0:P, 512:1024])
            nc.sync.dma_start(out=vec_sb[:], in_=vec_d[:])
            nc.sync.dma_start(out=avg_sb[:], in_=avg_d[:])
            for kt in range(1, KT):
                nc.sync.dma_start(out=x_sb[:, kt, :], in_=x_d[kt * P:(kt + 1) * P, :])
            nc.sync.dma_start(out=wq_sb[:], in_=wq_d[:])
            nc.sync.dma_start(out=wk_sb[:], in_=wk_d[:])
            nc.sync.dma_start(out=wv_sb[:], in_=wv_d[:])
            nc.sync.dma_start(out=wp_sb[:], in_=wp_d[:])

            with tc.tile_pool(name="ps_g", bufs=2, space="PSUM") as ps_g:
                # PE warmth: cheap small fp8 DR matmuls into a scratch bank
                warm_ps = ps_g.tile([P, P], f32, tag="warmps")

                def warm(k):
                    for _ in range(k):
                        nc.tensor.matmul(warm_ps, ones_sb[:], warm_sb[:],
                                         start=True, stop=True, perf_mode=DR)

                warm(12)

                # ---- GroupNorm: per-channel (mean, E[x^2]) -> group stats
                # via block-averaging matmul -> scale/shift -> n in fp8 ----
                for kt in range(KT):
                    st = small.tile([P, 2], f32, tag="st")  # mean | E[x^2]
                    if kt < 2:  # DVE bn_stats path
                        bstats = small.tile([P, 2, 6], f32, tag="bstats")
                        mv = small.tile([P, 2], f32, tag="mv")
                        nc.vector.bn_stats(out=bstats[:, 0, :], in_=x_sb[:, kt, 0:512])
                        nc.vector.bn_stats(out=bstats[:, 1, :], in_=x_sb[:, kt, 512:1024])
                        nc.vector.bn_aggr(out=mv, in_=bstats)
                        nc.vector.tensor_copy(st[:, 0:1], mv[:, 0:1])
                        nc.vector.scalar_tensor_tensor(
                            out=st[:, 1:2], in0=mv[:, 0:1], scalar=mv[:, 0:1],
                            in1=mv[:, 1:2], op0=Alu.mult, op1=Alu.add,
                        )
                    else:  # ACT accumulator path
                        nc.scalar.activation(out=scr_sb, in_=x_sb[:, kt, :],
                                             func=Act.Identity, bias=zero_sb,
                                             scale=1.0 / N, accum_out=st[:, 0:1])
                        nc.scalar.activation(out=scr_sb, in_=x_sb[:, kt, :],
                                             func=Act.Square, bias=zero_sb,
                                             scale=1.0 / np.sqrt(N),
                                             accum_out=st[:, 1:2])

                    # group aggregate+broadcast in one small matmul
                    b_ps = ps_g.tile([P, 2], f32, tag="gmm")
                    nc.tensor.matmul(b_ps, avg_sb, st, start=True, stop=True)
                    warm(7)
                    bc = small.tile([P, 2], f32, tag="bc")
                    nc.scalar.copy(bc, b_ps)
                    mean = bc[:, 0:1]
                    vneg = small.tile([P, 1], f32, tag="vneg")
                    nc.vector.scalar_tensor_tensor(
                        out=vneg, in0=mean, scalar=mean, in1=bc[:, 1:2],
                        op0=Alu.mult, op1=Alu.subtract,  # mean^2 - E[x^2]
                    )
                    var = small.tile([P, 1], f32, tag="var")
                    nc.vector.tensor_scalar_mul(var, vneg, -1.0)
                    sd = small.tile([P, 1], f32, tag="sd")
                    nc.scalar.activation(out=sd, in_=var, func=Act.Sqrt,
                                         bias=eps_sb, scale=1.0)
                    rstd = small.tile([P, 1], f32, tag="rstd")
                    nc.vector.reciprocal(rstd, sd)
                    gsc = small.tile([P, 1], f32, tag="gsc")
                    nc.vector.tensor_mul(gsc, rstd, gnw_sb[:, kt:kt + 1])
                    gscn = small.tile([P, 1], f32, tag="gscn")
                    nc.vector.tensor_scalar_mul(gscn, gsc, -1.0)
                    gshp = small.tile([P, 1], f32, tag="gshp")  # gnb - mean*gsc
                    nc.vector.scalar_tensor_tensor(
                        out=gshp, in0=mean, scalar=gscn, in1=gnb_sb[:, kt:kt + 1],
                        op0=Alu.mult, op1=Alu.add,
                    )
                    # n8 = x*gsc + gshp  (ACT, fp8 out)
                    nc.scalar.activation(out=n_sb[:, kt, :], in_=x_sb[:, kt, :],
                                         func=Act.Identity, bias=gshp,
                                         scale=gsc)
                    # xbp = x + bp' in the same window (DVE, SBUF 2x mode)
                    nc.vector.tensor_scalar(
                        out=xbp_sb[:, kt, :], in0=x_sb[:, kt, :],
                        scalar1=bp_sb[:, kt:kt + 1], scalar2=None,
                        op0=Alu.add)

                warm(7)
                # preload the exp table while the QKV matmuls stream
                nc.scalar.activation(out=dummy_sb, in_=x_sb[:, 0, 0:1], func=Act.Exp,
                                     bias=zero_sb, scale=1.0)

            with tc.tile_pool(name="ps_qkv", bufs=4, space="PSUM") as ps_qkv:
                # ---- Q projection (DoubleRow, weights stationary) ----
                for dt in range(KT):
                    mm = ps_qkv.tile([P, N], f32, tag="mm", name=f"q{dt}")
                    for ktp in range(2):
                        for nh in range(2):
                            nc.tensor.matmul(
                                mm[:, nh * 512:(nh + 1) * 512],
                                wq_sb[:, 2 * ktp:2 * ktp + 2, dt * P:(dt + 1) * P],
                                n_sb[:, 2 * ktp:2 * ktp + 2, nh * 512:(nh + 1) * 512],
                                start=(ktp == 0), stop=(ktp == 1), perf_mode=DR,
                            )
                    # q8 = raw/16 + 4*bq  (= 4*q_true), ACT evict
                    nc.scalar.activation(out=q_sb[:, dt, :], in_=mm,
                                         func=Act.Identity,
                                         bias=bq_sb[:, dt:dt + 1], scale=1.0 / 16)

                # ---- K projection: evict on DVE to balance engines ----
                for dt in range(KT):
                    mm = ps_qkv.tile([P, N], f32, tag="mm", name=f"k{dt}")
                    for ktp in range(2):
                        for nh in range(2):
                            nc.tensor.matmul(
                                mm[:, nh * 512:(nh + 1) * 512],
                                wk_sb[:, 2 * ktp:2 * ktp + 2, dt * P:(dt + 1) * P],
                                n_sb[:, 2 * ktp:2 * ktp + 2, nh * 512:(nh + 1) * 512],
                                start=(ktp == 0), stop=(ktp == 1), perf_mode=DR,
                            )
                    nc.vector.tensor_scalar(
                        out=k_sb[:, dt, :], in0=mm, scalar1=1.0 / 16,
                        scalar2=bk_sb[:, dt:dt + 1], op0=Alu.mult, op1=Alu.add)

                # ---- V transposed: vT[m, c] (n stationary, wv moving);
                # v bias folds into bp' on host ----
                for mg in range(KT):  # 2 m-tiles per psum tile
                    mm = ps_qkv.tile([P, N], f32, tag="mm", name=f"v{mg}")
                    for ml in range(2):
                        mt = 2 * mg + ml
                        for ktp in range(2):
                            nc.tensor.matmul(
                                mm[:, ml * 512:(ml + 1) * 512],
                                n_sb[:, 2 * ktp:2 * ktp + 2, mt * P:(mt + 1) * P],
                                wv_sb[:, 2 * ktp:2 * ktp + 2, :],
                                start=(ktp == 0), stop=(ktp == 1), perf_mode=DR,
                            )
                    nc.scalar.activation(
                        out=vT_sb[:, 2 * mg:2 * mg + 2, :],
                        in_=mm.rearrange("p (g c) -> p g c", g=2),
                        func=Act.Identity, bias=zero_sb, scale=1.0 / 16)

            # ---- scores transposed + exp, pipelined per 2 m-tiles ----
            # sT[m, n] = sum_c k[c, m] q[c, n]; exp on ACT -> fp8 eT
            with tc.tile_pool(name="ps_s", bufs=2, space="PSUM") as ps_s:
                for mtp in range(4):
                    s_ps = ps_s.tile([P, 2, N], f32, tag="s", name=f"s{mtp}")
                    for ml in range(2):
                        mt = 2 * mtp + ml
                        for ktp in range(2):
                            for nh in range(2):
                                nc.tensor.matmul(
                                    s_ps[:, ml, nh * 512:(nh + 1) * 512],
                                    k_sb[:, 2 * ktp:2 * ktp + 2, mt * P:(mt + 1) * P],
                                    q_sb[:, 2 * ktp:2 * ktp + 2, nh * 512:(nh + 1) * 512],
                                    start=(ktp == 0), stop=(ktp == 1), perf_mode=DR,
                                )
                    # raw = 16*s_true; exp(SCALE/16 * raw) in [~0.1, ~8]
                    nc.scalar.activation(out=eT_sb[:, 2 * mtp:2 * mtp + 2, :],
                                         in_=s_ps, func=Act.Exp,
                                         bias=zero_sb, scale=SCALE / 16)

            with (
                tc.tile_pool(name="ps_sum", bufs=1, space="PSUM") as ps_sum,
                tc.tile_pool(name="ps_av", bufs=2, space="PSUM") as ps_av,
            ):
                # ---- softmax denominators, broadcast to all partitions ----
                sum_ps = ps_sum.tile([P, N], f32, tag="sum")
                for mtp in range(4):
                    for nh in range(2):
                        nc.tensor.matmul(
                            sum_ps[:, nh * 512:(nh + 1) * 512],
                            ones_sb[:],
                            eT_sb[:, 2 * mtp:2 * mtp + 2, nh * 512:(nh + 1) * 512],
                            start=(mtp == 0), stop=(mtp == 3), perf_mode=DR,
                        )
                nc.vector.reciprocal_approx_fast(out=rsr_sb, in_=sum_ps)

                # ---- out[c, n] = (sum_m vT[m,c] eT[m,n]) / rowsum[n] ----
                for ct in range(KT):
                    mm = ps_av.tile([P, N], f32, tag="mm", name=f"av{ct}")
                    for mtp in range(4):
                        for nh in range(2):
                            nc.tensor.matmul(
                                mm[:, nh * 512:(nh + 1) * 512],
                                vT_sb[:, 2 * mtp:2 * mtp + 2, ct * P:(ct + 1) * P],
                                eT_sb[:, 2 * mtp:2 * mtp + 2, nh * 512:(nh + 1) * 512],
                                start=(mtp == 0), stop=(mtp == 3), perf_mode=DR,
                            )
                    # o8 = raw * rsr = 4*attnout_true (DVE)
                    nc.vector.tensor_tensor(out=o_sb[:, ct, :], in0=mm,
                                            in1=rsr_sb, op=Alu.mult)

            with tc.tile_pool(name="ps_pr", bufs=2, space="PSUM") as ps_pr:
                # ---- final projection + residual, stream out ----
                for dt in range(KT):
                    mm = ps_pr.tile([P, N], f32, tag="mm", name=f"p{dt}")
                    for ktp in range(2):
                        for nh in range(2):
                            nc.tensor.matmul(
                                mm[:, nh * 512:(nh + 1) * 512],
                                wp_sb[:, 2 * ktp:2 * ktp + 2, dt * P:(dt + 1) * P],
                                o_sb[:, 2 * ktp:2 * ktp + 2, nh * 512:(nh + 1) * 512],
                                start=(ktp == 0), stop=(ktp == 1), perf_mode=DR,
                            )
                    y_sb = work.tile([P, N], f32, tag="y")
                    # y = raw/256 + (x + bp')
                    nc.vector.scalar_tensor_tensor(
                        out=y_sb, in0=mm, scalar=1.0 / 256,
                        in1=xbp_sb[:, dt, :], op0=Alu.mult, op1=Alu.add)
                    nc.sync.dma_start(out=y_d[dt * P:(dt + 1) * P, :], in_=y_sb)

    nc.finalize()
    return nc


def _get_nc():
    if "nc" not in _CACHE:
        _CACHE["nc"] = _build_bass()
    return _CACHE["nc"]


def _make_in_maps(x, gn_w, gn_b, q_w, q_b, k_w, k_b, v_w, v_b, p_w, p_b):
    import ml_dtypes
    f8 = ml_dtypes.float8_e4m3
    x = np.asarray(x, np.float32)
    B = x.shape[0]
    assert x.shape == (B, CH, 32, 32) and B == NCORES

    def pc(vec):  # [512] -> [128, 4] with c = t*128 + p
        return np.asarray(vec, np.float32).reshape(KT, P).T

    def w8(w):  # [Cout, Cin] -> fp8 [P, KT, Cout] of 64*w.T
        wt = np.asarray(w, np.float32).T * 64.0  # [Cin, Cout]
        return np.ascontiguousarray(
            wt.reshape(KT, P, CH).transpose(1, 0, 2).astype(f8))

    bp_fold = np.asarray(p_w, np.float32) @ np.asarray(v_b, np.float32) \
        + np.asarray(p_b, np.float32)
    vecs = np.concatenate(
        [pc(4.0 * np.asarray(q_b)), pc(4.0 * np.asarray(k_b)),
         pc(gn_w), pc(gn_b), pc(bp_fold)], axis=1
    )
    avg = np.kron(np.eye(2, dtype=np.float32),
                  np.full((64, 64), 1.0 / 64, np.float32))
    shared = {
        "wq8": w8(q_w),
        "wk8": w8(k_w),
        "wv8": w8(v_w),
        "wp8": w8(p_w),
        "vecs": np.ascontiguousarray(vecs),
        "avg": np.ascontiguousarray(avg),
    }
    return [
        dict(shared, x=np.ascontiguousarray(x[b].reshape(CH, N)))
        for b in range(B)
    ]


def _run(in_maps, **kwargs):
    from concourse.bass_utils import run_bass_kernel_spmd
    return run_bass_kernel_spmd(_get_nc(), in_maps, core_ids=list(range(NCORES)), **kwargs)


def kernel(**inputs):
    in_maps = _make_in_maps(**inputs)
    res = _run(in_maps)
    out = np.stack([r["y"].reshape(CH, 32, 32) for r in res.results], axis=0)
    return out.astype(np.float32)
